# revision 1
# baseline (speedup 1.0000x reference)
"""Trainium2 Bass kernel for per-edge-type linear routing (MoE-style).

Computes out[i] = W[type_i] @ x[i] + b[type_i] for N=131072 edges,
C=D=256, T=8 types, on 8 NeuronCores.

Strategy: expert-grouped data parallelism. On the host we stable-sort the
edges by type and deal them round-robin to the 8 cores, so every core gets
~N/8 edges grouped into 8 contiguous per-type segments (segment sizes are
shared across cores, padded to multiples of 128 -> identical SPMD program,
balanced for any type distribution). Each core runs ONE dense GEMM pass
over its columns: for each 128-edge tile the stationary operand is the
transposed edge-feature tile and the moving operand is the type's
transposed weight matrix; bias is added by the DVE during the PSUM->SBUF
evict. This does 1x the required FLOPs (the reference's masked form does
T=8x).

Precision: matmul inputs and outputs are fp16 (accumulation stays fp32 in
PSUM). fp16 runs the PE at 1 cycle/row (2x the fp32 rate) and halves the
HBM traffic, for ~2x end-to-end; the added rounding error (~1e-3 rel
Frobenius) is far inside the 2e-2 gate. Bias stays fp32.

Device layout per core:
  xt   [256, NP]  fp16  edge features, transposed, type-grouped + padded
  wt   [8, 256, 256] fp16  wt[t] = W[t].T (contraction dim first)
  bias [8, 256]   fp32
  y    [NP, 256]  fp16  outputs in the same grouped order

The host then scatters valid rows of y back to the original edge order.
"""

import numpy as np

N_CORES = 8
T = 8
C = 256
D = 256
P = 128
BLK = 1024  # max columns per DMA block (the last block may be shorter)

_cache = {}


def _build_program(NP, tile_type, repeats=1):
    """Build + compile the SPMD Bass program for one core.

    NP: padded number of edge columns (multiple of BLK).
    tile_type: tuple of per-128-column-tile type ids, len NP // P.
    repeats: >1 wraps the body in a hardware loop (timing harness only).
    """
    import contextlib

    import concourse.tile as tile
    from concourse import bacc, mybir

    f32 = mybir.dt.float32
    f16 = mybir.dt.float16
    nc = bacc.Bacc("TRN2", target_bir_lowering=False, debug=False)

    xt = nc.dram_tensor("xt", [C, NP], f16, kind="ExternalInput")
    wt = nc.dram_tensor("wt", [T, C, D], f16, kind="ExternalInput")
    bias = nc.dram_tensor("bias", [T, D], f32, kind="ExternalInput")
    chain = nc.dram_tensor("chain", [1, 4], f32, kind="ExternalInput")
    # y is laid out [partition, tile, D] so each out-DMA writes one fully
    # contiguous run per partition (tiles*D*2 bytes) instead of `tiles`
    # strided 512B segments; the host untangles tile-major order for free.
    y = nc.dram_tensor("y", [P, NP // P, D], f16, kind="ExternalOutput")
    chain_out = nc.dram_tensor("chain_out", [1, 4], f32, kind="ExternalOutput")

    # 1024-column blocks (256 KB per input DMA); short last block instead of
    # padding NP up to a BLK multiple.
    blocks = []
    c = 0
    while c < NP:
        b = min(BLK, NP - c)
        blocks.append((c, b))
        c += b

    with tile.TileContext(nc) as tc:
        with (
            tc.tile_pool(name="wpool", bufs=1) as wpool,
            tc.tile_pool(name="xpool", bufs=6) as xpool,
            tc.tile_pool(name="opool", bufs=6) as opool,
            tc.tile_pool(name="pspool", bufs=8, space="PSUM") as pspool,
        ):
            # Tiny passthrough so a timing harness can chain executions.
            cht = wpool.tile([1, 4], f32, name="cht", tag="cht")
            nc.sync.dma_start(out=cht[:], in_=chain[:])
            nc.sync.dma_start(out=chain_out[:], in_=cht[:])

            loop_ctx = (
                tc.For_i(0, repeats) if repeats > 1 else contextlib.nullcontext()
            )

            # Weight / bias tiles are loaded lazily right before first use so
            # they do not delay the first x blocks on the DMA engines.
            wtiles = {}

            def ensure_w(t):
                if t in wtiles:
                    return
                halves = []
                for h in range(2):
                    w_ = wpool.tile([P, D], f16, name=f"w{t}_{h}", tag=f"w{t}_{h}")
                    nc.sync.dma_start(out=w_[:], in_=wt[t, h * P:(h + 1) * P, :])
                    halves.append(w_)
                # bias row broadcast across partitions; added during evict
                bt = wpool.tile([P, D], f32, name=f"b{t}", tag=f"b{t}")
                nc.sync.dma_start(
                    out=bt[:], in_=bias[t:t + 1, :].to_broadcast((P, D))
                )
                wtiles[t] = (halves, bt)

            with loop_ctx:
                for c0, b in blocks:
                    tiles_per_blk = b // P
                    xb0 = xpool.tile([P, b], f16, name="xb0", tag="xb0")
                    xb1 = xpool.tile([P, b], f16, name="xb1", tag="xb1")
                    nc.sync.dma_start(out=xb0[:], in_=xt[0:P, c0:c0 + b])
                    nc.sync.dma_start(out=xb1[:], in_=xt[P:C, c0:c0 + b])
                    ost = opool.tile([P, tiles_per_blk * D], f16, name="ost", tag="ost")
                    for j in range(tiles_per_blk):
                        t = tile_type[c0 // P + j]
                        ensure_w(t)
                        halves, bt = wtiles[t]
                        ps = pspool.tile([P, D], f32, name="ps", tag="ps")
                        nc.tensor.matmul(
                            ps[:], xb0[:, j * P:(j + 1) * P], halves[0][:],
                            start=True, stop=False,
                        )
                        nc.tensor.matmul(
                            ps[:], xb1[:, j * P:(j + 1) * P], halves[1][:],
                            start=False, stop=True,
                        )
                        nc.vector.tensor_tensor(
                            ost[:, j * D:(j + 1) * D], ps[:], bt[:],
                            op=mybir.AluOpType.add,
                        )
                    nc.sync.dma_start(
                        out=y[:, c0 // P:c0 // P + tiles_per_blk, :],
                        in_=ost[:],
                    )

    nc.compile()
    return nc


def _plan(ids):
    """Shared sharding plan: returns (core_idx, offs, G, NP_pad, tile_type)."""
    order = np.argsort(ids, kind="stable")
    core_idx = [order[k::N_CORES] for k in range(N_CORES)]
    cnts = np.stack(
        [np.bincount(ids[ci], minlength=T)[:T].astype(np.int64)
         for ci in core_idx]
    )
    gmax = cnts.max(axis=0)
    G = ((gmax + P - 1) // P) * P
    NP = int(G.sum())
    NP_pad = NP  # blocks handle any 128-multiple; no BLK rounding needed
    offs = np.concatenate([[0], np.cumsum(G)]).astype(np.int64)
    tile_type = []
    for t in range(T):
        tile_type += [t] * (int(G[t]) // P)
    tile_type += [0] * ((NP_pad - NP) // P)
    return core_idx, offs, tuple(tile_type), NP_pad


def _make_in_maps(inputs, plan):
    """Build per-core device input maps + the scatter info to unshard y."""
    core_idx, offs, tile_type, NP_pad = plan
    x = np.asarray(inputs["edge_features"], dtype=np.float32)
    w = np.asarray(inputs["weights"], dtype=np.float32)
    b = np.ascontiguousarray(np.asarray(inputs["biases"], dtype=np.float32))
    ids = np.asarray(inputs["edge_type_ids"])

    wt_full = np.ascontiguousarray(
        w.transpose(0, 2, 1).astype(np.float16))  # [T, C, D] fp16
    chain0 = np.zeros((1, 4), dtype=np.float32)
    in_maps = []
    seg_rows = []
    for k in range(N_CORES):
        ci = core_idx[k]
        ids_k = ids[ci]
        xr = np.zeros((NP_pad, C), dtype=np.float16)
        segs = []
        for t in range(T):
            idx_t = ci[ids_k == t]
            cnt = idx_t.shape[0]
            if cnt:
                xr[offs[t]:offs[t] + cnt] = x[idx_t]
            segs.append((int(offs[t]), cnt, idx_t))
        seg_rows.append(segs)
        in_maps.append({
            "xt": np.ascontiguousarray(xr.T),
            "wt": wt_full,
            "bias": b,
            "chain": chain0,
        })
    return in_maps, seg_rows


def kernel(edge_features, weights, biases, edge_type_ids):
    from concourse.bass_utils import run_bass_kernel_spmd

    inputs = {
        "edge_features": edge_features,
        "weights": weights,
        "biases": biases,
        "edge_type_ids": edge_type_ids,
    }
    ids = np.asarray(edge_type_ids)
    n = np.asarray(edge_features).shape[0]

    plan = _plan(ids)
    core_idx, offs, tile_type, NP_pad = plan

    key = (NP_pad, tile_type)
    if key not in _cache:
        _cache[key] = _build_program(NP_pad, tile_type)
    nc = _cache[key]

    in_maps, seg_rows = _make_in_maps(inputs, plan)

    res = run_bass_kernel_spmd(nc, in_maps, list(range(N_CORES)))

    # zeros, not empty: rows whose type id falls outside [0, T) are never
    # written by any segment, and the reference leaves them at zero too
    out = np.zeros((n, D), dtype=np.float32)
    for k in range(N_CORES):
        yk = res.results[k]["y"]  # [P, NP//P, D], tile-major device layout
        yg = yk.transpose(1, 0, 2).reshape(-1, D)  # column-order [NP, D]
        for off, cnt, idx_t in seg_rows[k]:
            if cnt:
                out[idx_t] = yg[off:off + cnt].astype(np.float32)
    return out



# revision 19
# speedup vs baseline: 37.1935x; 37.1935x over previous
"""Trainium2 Bass kernel for per-edge-type linear routing (MoE-style).

Computes out[i] = W[type_i] @ x[i] + b[type_i] for N=131072 edges,
C=D=256, T=8 types, on 8 NeuronCores.

Strategy: TYPE-parallel expert placement. With T=8 types on 8 cores, core
k owns every edge of type k (the host routes edges by type, free). Each
core then needs only its OWN 128KB weight slice instead of the full 1MB
[T,D,C] stack — HBM traffic is the kernel's roofline, so shrinking the
weight load is a direct win over plain data-parallelism. The uniform
type distribution keeps edge counts within ~2% across cores, and since
every core runs the IDENTICAL program over NP padded columns (shorter
types just carry more zero columns), all cores take the same time. This
does 1x the required FLOPs (the reference's masked form does T=8x).

GEMM orientation: the WEIGHTS are the stationary operand and the edge
features stream as the moving operand, out[d, e] = sum_c W[k].T[c, d]
* x[c, e]. Per group of up to 512 edges this takes 4 matmuls (2
contraction halves x 2 output halves, N = group size), so LDWEIGHTS
traffic is tiny and each matmul streams up to 512 columns. Outputs land
in PSUM as [D-half, edges], which makes the bias a PER-PARTITION scalar:
the d0 half is evicted by the Vector engine (tensor_scalar add) and the
d1 half by the Activation engine (activation Identity with bias),
splitting the ~40us of PSUM-evict work across two engines.

Precision: matmul inputs and outputs are fp16 (accumulation stays fp32 in
PSUM). fp16 runs the PE at 1 cycle/row (2x the fp32 rate) and halves the
HBM traffic; the added rounding error (~1e-3 rel Frobenius) is far inside
the 2e-2 gate.

DMA layout: the dominant per-DMA cost on TRN2 is the ~650ns sequencer +
~625ns HWDGE issue overhead, so data moves in few, large transfers:
  xt   [128, 2, NP] fp16  [p, h, c] = feature h*128+p of core-local edge c
                          -> ONE DMA per 2048-edge block (1MB, two 4KB
                          contiguous runs per partition).
  wt   [128, 512]   fp16  this core's 4 weight blocks (k, d) -> ONE DMA.
  bias [128, 2]     f32   this core's bias, per-partition -> ONE tiny DMA.
  y    [128, 2, NP] fp16  [p, d, c] = out dim d*128+p of core-local edge c
                          -> ONE DMA per block (two 4KB runs/partition).
x loads issue from the SP HWDGE, y stores from the (otherwise idle)
GpSimd SWDGE so a store waiting on its block's evicts never head-of-line
blocks the next block's x load on the in-order SP sequencer.

The host then scatters rows of y back to the original edge order.
"""

import numpy as np

N_CORES = 8
T = 8
C = 256
D = 256
P = 128
BLK = 2048   # edge columns per DMA block (the last block may be shorter)
GRP = 512    # max edge columns per matmul group (PSUM bank = 512 fp32)

_cache = {}


def _build_program(NP, repeats=1):
    """Build + compile the SPMD Bass program for one core.

    NP: padded number of edge columns (multiple of P), same on all cores.
    repeats: >1 wraps the body in a hardware loop (timing harness only).
    """
    import contextlib

    import concourse.tile as tile
    from concourse import bacc, mybir

    f32 = mybir.dt.float32
    f16 = mybir.dt.float16
    nc = bacc.Bacc("TRN2", target_bir_lowering=False, debug=False)

    xt = nc.dram_tensor("xt", [P, 2, NP], f16, kind="ExternalInput")
    wt = nc.dram_tensor("wt", [P, 2 * 2 * P], f16, kind="ExternalInput")
    bias = nc.dram_tensor("bias", [P, 2], f32, kind="ExternalInput")
    chain = nc.dram_tensor("chain", [1, 4], f32, kind="ExternalInput")
    y = nc.dram_tensor("y", [P, 2, NP], f16, kind="ExternalOutput")
    chain_out = nc.dram_tensor("chain_out", [1, 4], f32, kind="ExternalOutput")

    blocks = []
    c = 0
    while c < NP:
        b = min(BLK, NP - c)
        blocks.append((c, b))
        c += b

    def wslice(wall, k, d):
        off = (k * 2 + d) * P
        return wall[:, off:off + P]

    with tile.TileContext(nc) as tc:
        with (
            tc.tile_pool(name="wpool", bufs=1) as wpool,
            tc.tile_pool(name="xpool", bufs=4) as xpool,
            tc.tile_pool(name="opool", bufs=4) as opool,
            tc.tile_pool(name="pspool", bufs=8, space="PSUM") as pspool,
        ):
            loop_ctx = (
                tc.For_i(0, repeats) if repeats > 1 else contextlib.nullcontext()
            )

            with loop_ctx:
                # This core's weights in one DMA; bias in one tiny DMA.
                wall = wpool.tile([P, 2 * 2 * P], f16, name="wall", tag="wall")
                nc.sync.dma_start(out=wall[:], in_=wt[:, :])
                ball = wpool.tile([P, 2], f32, name="ball", tag="ball")
                nc.sync.dma_start(out=ball[:], in_=bias[:, :])

                for c0, b in blocks:
                    xb = xpool.tile([P, 2, b], f16, name="xb", tag="xb")
                    nc.sync.dma_start(
                        out=xb[:], in_=xt[:, :, c0:c0 + b],
                    )
                    ost = opool.tile([P, 2, b], f16, name="ost", tag="ost")
                    for o in range(0, b, GRP):
                        n = min(GRP, b - o)
                        for d in range(2):
                            ps = pspool.tile([P, GRP], f32, name="ps",
                                             tag="ps")
                            nc.tensor.matmul(
                                ps[:, :n], wslice(wall, 0, d),
                                xb[:, 0, o:o + n],
                                start=True, stop=False,
                            )
                            nc.tensor.matmul(
                                ps[:, :n], wslice(wall, 1, d),
                                xb[:, 1, o:o + n],
                                start=False, stop=True,
                            )
                            ob = ost[:, d, o:o + n]
                            bcol = ball[:, d:d + 1]
                            if d == 0:
                                nc.vector.tensor_scalar(
                                    ob, ps[:, :n], bcol, None,
                                    op0=mybir.AluOpType.add,
                                )
                            else:
                                nc.scalar.activation(
                                    ob, ps[:, :n],
                                    mybir.ActivationFunctionType.Identity,
                                    bias=bcol, scale=1.0,
                                )
                    # Store via the (otherwise idle) GpSimd SWDGE so the
                    # in-order SP sequencer only issues loads.
                    nc.gpsimd.dma_start(
                        out=y[:, :, c0:c0 + b], in_=ost[:],
                    )

            # Tiny passthrough so a timing harness can chain executions.
            # Emitted last (and chain_out via GpSimd) so the ~2us chain-in
            # DMA round trip never head-of-line blocks a real load on SP.
            cht = wpool.tile([1, 4], f32, name="cht", tag="cht")
            nc.sync.dma_start(out=cht[:], in_=chain[:])
            nc.gpsimd.dma_start(out=chain_out[:], in_=cht[:])

    nc.compile()
    return nc


def _plan(ids):
    """Type-parallel plan: returns (core_idx, NP_pad).

    core_idx[k] = original indices of the edges core k owns (= type k),
    in stable order. NP_pad = max count over cores, padded to 128.
    """
    core_idx = [np.nonzero(ids == k)[0] for k in range(N_CORES)]
    maxcnt = max(int(ci.shape[0]) for ci in core_idx)
    NP = ((max(maxcnt, 1) + P - 1) // P) * P
    return core_idx, NP


def _make_in_maps(inputs, plan):
    """Build per-core device input maps."""
    core_idx, NP_pad = plan
    x = np.asarray(inputs["edge_features"], dtype=np.float32)
    w = np.asarray(inputs["weights"], dtype=np.float32)
    b = np.asarray(inputs["biases"], dtype=np.float32)

    wt_full = w.transpose(0, 2, 1).astype(np.float16)   # [T, C, D]
    chain0 = np.zeros((1, 4), dtype=np.float32)
    in_maps = []
    for k in range(N_CORES):
        idx_k = core_idx[k]
        cnt = idx_k.shape[0]
        xr = np.zeros((NP_pad, C), dtype=np.float16)
        if cnt:
            xr[:cnt] = x[idx_k]
        # [NP, C] -> [C, NP] -> [2, 128, NP] -> [128, 2, NP]
        xt_dev = np.ascontiguousarray(
            xr.T.reshape(2, P, NP_pad).transpose(1, 0, 2)
        )
        # wt_dev[:, (k_half*2+d)*P : +P] = W[k].T[k_half*128:.., d*128:..]
        wt_dev = np.ascontiguousarray(
            wt_full[k % T].reshape(2, P, 2, P)           # kh kp d dp
            .transpose(1, 0, 2, 3)                        # kp kh d dp
            .reshape(P, 2 * 2 * P)
        )
        bias_dev = np.ascontiguousarray(b[k % T].reshape(2, P).T)  # [P, 2]
        in_maps.append({
            "xt": xt_dev,
            "wt": wt_dev,
            "bias": bias_dev,
            "chain": chain0,
        })
    return in_maps


def kernel(edge_features, weights, biases, edge_type_ids):
    from concourse.bass_utils import run_bass_kernel_spmd

    inputs = {
        "edge_features": edge_features,
        "weights": weights,
        "biases": biases,
        "edge_type_ids": edge_type_ids,
    }
    ids = np.asarray(edge_type_ids)
    n = np.asarray(edge_features).shape[0]

    plan = _plan(ids)
    core_idx, NP_pad = plan

    key = NP_pad
    if key not in _cache:
        _cache[key] = _build_program(NP_pad)
    nc = _cache[key]

    in_maps = _make_in_maps(inputs, plan)

    res = run_bass_kernel_spmd(nc, in_maps, list(range(N_CORES)))

    # zeros, not empty: rows whose type id falls outside [0, T) are never
    # assigned to a core, and the reference leaves them at zero too
    out = np.zeros((n, D), dtype=np.float32)
    for k in range(N_CORES):
        idx_k = core_idx[k]
        cnt = idx_k.shape[0]
        if not cnt:
            continue
        yk = res.results[k]["y"]  # [P, 2, NP] device layout
        yg = yk.transpose(2, 1, 0).reshape(-1, D)  # [NP, 256] local order
        out[idx_k] = yg[:cnt].astype(np.float32)
    return out
